# revision 9
# baseline (speedup 1.0000x reference)
"""LIF spiking-neuron recurrence kernel for Trainium2 (8 NeuronCores, SPMD).

Problem: x [32, 100, 8192] f32, decay [1] f32.
    d = sigmoid(decay)
    mem_0 = x[:,0];  mem_t = mem_{t-1} * d * (1 - spike_{t-1}) + x[:,t]
    spike_t = (mem_t > 0.5);  out[:,t] = spike_t  (f32 0/1)

Device formulation (bit-exact vs the reference):
    W_{-1} = 0
    M_t = (W_{t-1} * d) + x_t
    W_t = (M_t <= 0.5) * M_t
spike_t = (M_t > 0.5) = (W_t == 0) exactly. The recurrence runs as ONE
custom DVE op per chunk (out AP trails in0 by F elements in the same
persistent W buffer).

Output format: spikes are BITPACKED 8 timesteps per byte before leaving
the device, so the store is 0.41 MB/core instead of 3.3 MB/core (the
whole kernel is HBM-bound: 13.1 MB of x load per core dominates).
Byte (p, g*256+f) bit k = spike at t=8g+k; groups g=0..11 are full,
group 12 carries t=96..99 in bits 0..3. Host unpacks with np.unpackbits.

Bit-plane extraction is split across engines:
  - ACT (scalar engine): planes 0..ACT_PLANES-1 of the early groups via
    q = Square(1e19*W) (0 -> 0, nonzero -> >=1 or inf) then
    t_k = Relu(2^k - 2^k*q) which is exactly 2^k or 0, written as uint8.
  - DVE: remaining planes via tensor_scalar is_equal*2^k (2x mode), and
    ALL planes of the last window + remainder (tail latency: those groups
    only exist after the final loads land, so they take the fast path).
Bit-planes are combined with 7 tensor_tensor ADDs per window on uint16
bitcast views (2 packed bytes per slot at 2x mode = 4 bytes/cycle;
disjoint bits mean sums stay < 2^16 so the fp32-internal ALU is exact).

Sharding: d-shard as before: core c owns d in [1024c, 1024c+1024);
per-core layout [128, T*256] with partition p = b*4 + (d_local//256).
No cross-core communication.
"""

from contextlib import ExitStack

import numpy as np

N_CORES = 8
B, T, D = 32, 100, 8192
P = 128          # SBUF partitions
F = 256          # free elements per timestep per core
THRESH = 0.5
NG_FULL = T // 8          # 12 full 8-step groups
REM = T - NG_FULL * 8     # 4 remainder steps (bits 0..3 of group 12)
NG = NG_FULL + 1          # 13 output groups
OUT_F = NG * F            # 3328 output bytes per partition

# ---- schedule / engine-split config ------------------------------------
# Load chunks in timesteps. 8-multiples so spike windows align to groups;
# small chunks at the end keep the post-load-floor tail short.
SCHED = [8, 8, 16, 16, 16, 16, 8, 4, 4, 4]
# Spike windows in (first_group, n_groups). Windows 0..4 use the ACT
# assist; the last window and the remainder run entirely on DVE.
WINDOWS = [(0, 2), (2, 2), (4, 2), (6, 2), (8, 2), (10, 2)]
ACT_WINDOWS = 5          # windows [0..ACT_WINDOWS) get ACT assist
ACT_PLANES = 5           # planes 0..4 on ACT in assisted windows
MAXG = max(ng for _, ng in WINDOWS)

_BUILD_CACHE: dict = {}
_LIF_OP = None


def _get_lif_op():
    """Register the fused LIF-step custom DVE op (idempotent)."""
    global _LIF_OP
    if _LIF_OP is not None:
        return _LIF_OP
    from concourse.dve_ops import (
        CUSTOM_DVE_SPECS, OPS, _SUB_OPCODE_FOR_NAME, DveOp,
    )
    from concourse.dve_spec import C0, C1, Spec, Src0, Src1, lower
    from concourse.dve_table_gen import dve_ver_for
    from concourse.dve_uop import DveOpSpec

    name = "LIF_STEP_ANT"
    if name in _SUB_OPCODE_FOR_NAME:
        _LIF_OP = next(op for op in OPS if op.name == name)
        return _LIF_OP

    M = Src0 * C0 + Src1

    def _ref(in0, in1, s0, s1, imm2):
        m = (in0.astype(np.float32) * np.float32(s0)
             + in1.astype(np.float32)).astype(np.float32)
        return np.where(m <= np.float32(s1), m, np.float32(0.0)).astype(np.float32)

    spec = Spec(body=M * (M <= C1), reference=_ref)
    row = max(_SUB_OPCODE_FOR_NAME.values()) + 1
    assert row < 0x20
    _SUB_OPCODE_FOR_NAME[name] = row
    shas = {}
    for ver in ("v3",):  # TRN2
        tmp = DveOpSpec(name=name, opcode=row, uops=lower(spec, ver=ver),
                        rd1_en=True)
        shas[ver] = tmp.sha(ver)
    assert dve_ver_for("TRN2") == "v3"
    op = DveOp(name, spec, subdim=False, uops_sha=shas)
    OPS.append(op)
    CUSTOM_DVE_SPECS[name] = spec
    _LIF_OP = op
    return op


def _build_nc(t_steps: int, d_imm: float):
    import concourse.tile as tile
    from concourse import bacc, mybir

    assert t_steps == T, "schedule is hardcoded for T=100"
    lif_op = _get_lif_op()
    assert sum(SCHED) == T
    ends = []
    s = 0
    for tc in SCHED:
        s += tc
        ends.append(s)

    AF = mybir.ActivationFunctionType
    ALU = mybir.AluOpType

    nc = bacc.Bacc("TRN2", debug=False, target_bir_lowering=False)
    x_in = nc.dram_tensor("x", [P, T * F], mybir.dt.float32,
                          kind="ExternalInput")
    s_out = nc.dram_tensor("s", [P, OUT_F], mybir.dt.uint8,
                           kind="ExternalOutput")

    # Const APs for the per-plane Relu biases (2^k); only 0.0/1.0 ship
    # pre-registered.
    for k in range(1, 8):
        v = float(1 << k)
        key = (mybir.dt.float32, v)
        if key not in nc.const_aps.aps:
            ct = nc.alloc_sbuf_tensor(f"const-f32-{int(v)}", [P, 1],
                                      mybir.dt.float32)
            nc.gpsimd.memset(ct.ap(), v)
            nc.const_aps.aps[key] = ct.ap()
    nc.all_engine_barrier()

    with tile.TileContext(nc) as tcx, ExitStack() as ctx:
        xpool = ctx.enter_context(tcx.tile_pool(name="xp", bufs=3))
        qpool = ctx.enter_context(tcx.tile_pool(name="qp", bufs=1))
        kpool = ctx.enter_context(tcx.tile_pool(name="kp", bufs=3))
        tpool = ctx.enter_context(tcx.tile_pool(name="tp", bufs=2))
        spool = ctx.enter_context(tcx.tile_pool(name="sp", bufs=1))

        # Persistent state buffer: W[:, t*F:(t+1)*F] holds W_{t-1} (slot 0
        # is the zero initial state, slot t+1 is W_t).
        wbuf = spool.tile([P, (T + 1) * F], mybir.dt.float32)
        # Packed-spike accumulator, written window by window.
        acc = spool.tile([P, OUT_F], mybir.dt.uint8)
        # ACT warmup scratch (loads the activation table set during the
        # first DMA instead of on the critical path).
        scr = spool.tile([P, 32], mybir.dt.float32)

        nc.vector.memset(wbuf[:, 0:F], 0.0)
        nc.scalar.memzero(scr[:, :])
        nc.scalar.activation(out=scr[:, :], in_=scr[:, :], func=AF.Square)

        def w_view(g0, ng, nplanes=8):
            """[P, ng, nplanes*F] view of W for groups [g0, g0+ng): W_t
            for t = 8*(g0+g)+k sits at (g, k*F+f)."""
            a = wbuf[:, (8 * g0 + 1) * F:
                     (8 * g0 + 8 * (ng - 1) + nplanes + 1) * F]
            if ng == 1:
                return a.rearrange("p (g kf) -> p g kf", g=1)
            assert nplanes == 8
            return a.rearrange("p (g kf) -> p g kf", g=ng)

        # Per-window bit-plane tiles: created once per window, written by
        # the plane passes and consumed by the tree (same ring slot).
        wtk: dict = {}

        def plane_tiles(wkey, nplanes=8):
            if wkey not in wtk:
                wtk[wkey] = [
                    kpool.tile([P, MAXG * F], mybir.dt.uint8, tag=f"k{k}",
                               name=f"tk{k}_{wkey}")
                    for k in range(nplanes)
                ]
            return wtk[wkey]

        def emit_act_part(wkey, g0, ng):
            # q = Square(1e19 * W) for planes [0, ACT_PLANES) -> bf16
            wv = w_view(g0, ng)[:, :, 0:ACT_PLANES * F]
            qt = qpool.tile([P, MAXG * ACT_PLANES * F], mybir.dt.bfloat16,
                            tag="qt")
            qv = qt[:, :ng * ACT_PLANES * F].rearrange(
                "p (g kf) -> p g kf", g=ng)
            nc.scalar.activation(out=qv, in_=wv, func=AF.Square, scale=1e19)
            qk = qt[:, :ng * ACT_PLANES * F].rearrange(
                "p (g k f) -> p g k f", g=ng, k=ACT_PLANES)
            tks = plane_tiles(wkey)
            for k in range(ACT_PLANES):
                tv = tks[k][:, :ng * F].rearrange("p (g f) -> p g f", g=ng)
                w = float(1 << k)
                nc.scalar.activation(out=tv, in_=qk[:, :, k, :],
                                     func=AF.Relu, bias=w, scale=-w)

        def emit_dve_planes(wkey, g0, ng, planes):
            planes = list(planes)
            wv = w_view(g0, ng, nplanes=max(planes) + 1)
            tks = plane_tiles(wkey, nplanes=max(planes) + 1)
            for k in planes:
                tv = tks[k][:, :ng * F].rearrange("p (g f) -> p g f", g=ng)
                nc.vector.tensor_scalar(
                    out=tv, in0=wv[:, :, k * F:(k + 1) * F],
                    scalar1=0.0, scalar2=float(1 << k),
                    op0=ALU.is_equal, op1=ALU.mult)

        def u16(ap):
            return ap.bitcast(mybir.dt.uint16)

        def emit_tree(wkey, g0, ng, nplanes=8):
            n = ng * F
            tks = plane_tiles(wkey, nplanes=nplanes)
            outv = u16(acc[:, g0 * F:g0 * F + n])
            if nplanes == 8:
                ta = tpool.tile([P, MAXG * F // 2], mybir.dt.uint16, tag="ta")
                tb = tpool.tile([P, MAXG * F // 2], mybir.dt.uint16, tag="tb")
                tc_ = tpool.tile([P, MAXG * F // 2], mybir.dt.uint16, tag="tc")
                td = tpool.tile([P, MAXG * F // 2], mybir.dt.uint16, tag="td")
                a, b, c, d = (t[:, :n // 2] for t in (ta, tb, tc_, td))
                nc.vector.tensor_add(a, u16(tks[0][:, :n]), u16(tks[1][:, :n]))
                nc.vector.tensor_add(b, u16(tks[2][:, :n]), u16(tks[3][:, :n]))
                nc.vector.tensor_add(c, u16(tks[4][:, :n]), u16(tks[5][:, :n]))
                nc.vector.tensor_add(d, u16(tks[6][:, :n]), u16(tks[7][:, :n]))
                nc.vector.tensor_add(a, a, b)
                nc.vector.tensor_add(c, c, d)
                nc.vector.tensor_add(outv, a, c)
            else:
                assert nplanes == 4
                ta = tpool.tile([P, MAXG * F // 2], mybir.dt.uint16, tag="ta")
                tb = tpool.tile([P, MAXG * F // 2], mybir.dt.uint16, tag="tb")
                a, b = ta[:, :n // 2], tb[:, :n // 2]
                nc.vector.tensor_add(a, u16(tks[0][:, :n]), u16(tks[1][:, :n]))
                nc.vector.tensor_add(b, u16(tks[2][:, :n]), u16(tks[3][:, :n]))
                nc.vector.tensor_add(outv, a, b)

        # window index -> gate chunk index (last chunk whose LIF writes it)
        def gate_chunk(g0, ng):
            need = 8 * (g0 + ng)
            for ci, e in enumerate(ends):
                if e >= need:
                    return ci
            raise AssertionError

        act_at = {}      # chunk -> list of window idx for ACT emission
        planes_at = {}   # chunk -> list of window idx for DVE plane fills
        for wi, (g0, ng) in enumerate(WINDOWS):
            gc = gate_chunk(g0, ng)
            if wi < ACT_WINDOWS:
                act_at.setdefault(gc, []).append(wi)
                planes_at.setdefault(min(gc + 1, len(SCHED) - 1), []).append(wi)
        trees_at = {}    # chunk -> list of window idx (ACT windows only)
        for wi in range(ACT_WINDOWS - 1):
            gc = gate_chunk(*WINDOWS[wi])
            trees_at.setdefault(min(gc + 2, len(SCHED) - 1), []).append(wi)

        t0 = 0
        for ci, tc in enumerate(SCHED):
            xt = xpool.tile([P, 16 * F], mybir.dt.float32, tag="xt")
            nc.sync.dma_start(out=xt[:, :tc * F],
                              in_=x_in[:, t0 * F:(t0 + tc) * F])
            nc.vector._custom_dve(
                lif_op,
                out=wbuf[:, (t0 + 1) * F:(t0 + tc + 1) * F],
                in0=wbuf[:, t0 * F:(t0 + tc) * F],
                in1=xt[:, :tc * F],
                s0=d_imm, s1=THRESH)
            for wi in act_at.get(ci, []):
                emit_act_part(wi, *WINDOWS[wi])
            for wi in planes_at.get(ci, []):
                emit_dve_planes(wi, *WINDOWS[wi], range(ACT_PLANES, 8))
            for wi in trees_at.get(ci, []):
                emit_tree(wi, *WINDOWS[wi])
            t0 += tc

        # Tail: last window all-DVE, remainder, then the stall-prone tree
        # of the last ACT window (waits on ACT) at the very end.
        lw = ACT_WINDOWS  # index of the all-DVE last window
        emit_dve_planes(lw, *WINDOWS[lw], range(8))
        emit_dve_planes("rem", NG_FULL, 1, range(4))  # remainder planes
        emit_tree(lw, *WINDOWS[lw])
        emit_tree("rem", NG_FULL, 1, nplanes=4)
        emit_tree(ACT_WINDOWS - 1, *WINDOWS[ACT_WINDOWS - 1])

        # Stores: groups 0..7 unblock early; the rest goes at the end.
        nc.sync.dma_start(out=s_out[:, :8 * F], in_=acc[:, :8 * F])
        nc.sync.dma_start(out=s_out[:, 8 * F:], in_=acc[:, 8 * F:])
    nc.compile()
    return nc


def _get_nc(t_steps: int, d_imm: float):
    key = (t_steps, np.float32(d_imm).tobytes())
    if key not in _BUILD_CACHE:
        _BUILD_CACHE[key] = _build_nc(t_steps, d_imm)
    return _BUILD_CACHE[key]


def _shard_x(x: np.ndarray) -> list[np.ndarray]:
    b, t, d = x.shape
    # [b, t, core, chunk, 256] -> [core, b, chunk, t, 256] -> [core, 128, t*256]
    xr = x.reshape(b, t, N_CORES, 4, F).transpose(2, 0, 3, 1, 4)
    xr = np.ascontiguousarray(xr).reshape(N_CORES, P, t * F)
    return [xr[c] for c in range(N_CORES)]


def _unshard_spikes(s8: np.ndarray, t: int) -> np.ndarray:
    # s8: [core, 128, NG*256] packed bits; bit k of byte (p, g*256+f) is
    # spike at timestep 8g+k for lane (p, f).
    a = s8.reshape(N_CORES, P, NG, F, 1)
    bits = np.unpackbits(a, axis=-1, bitorder="little")  # [c, p, g, f, 8]
    bits = bits.transpose(0, 1, 2, 4, 3).reshape(N_CORES, P, NG * 8, F)
    bits = bits[:, :, :t, :]
    sr = bits.astype(np.float32).reshape(N_CORES, B, 4, t, F)
    sr = sr.transpose(1, 3, 0, 2, 4)
    return np.ascontiguousarray(sr).reshape(B, t, N_CORES * 4 * F)


def _sigmoid_f32(decay: np.ndarray) -> np.float32:
    import jax
    import jax.numpy as jnp
    d = np.asarray(jax.nn.sigmoid(jnp.asarray(decay, jnp.float32)))
    return np.float32(d.reshape(-1)[0])


def kernel(x: np.ndarray, decay: np.ndarray) -> np.ndarray:
    from concourse.bass_utils import run_bass_kernel_spmd

    x = np.asarray(x, dtype=np.float32)
    b, t, d = x.shape
    d_f32 = _sigmoid_f32(np.asarray(decay))

    nc = _get_nc(t, float(d_f32))
    shards = _shard_x(x)
    in_maps = [{"x": np.ascontiguousarray(s)} for s in shards]
    res = run_bass_kernel_spmd(nc, in_maps, core_ids=list(range(N_CORES)))
    s8 = np.stack([np.asarray(res.results[c]["s"]) for c in range(N_CORES)],
                  axis=0)
    return _unshard_spikes(s8, t)
